# revision 5
# baseline (speedup 1.0000x reference)
"""Trainium2 Bass kernel for nn_DMLNegHead (retrieval_knn head).

Strategy: data-parallel over batch B=16 across 8 NeuronCores (2 images per
core), prototype/offset params replicated. No collectives needed — every
output has a leading batch axis.

Per-core pipeline, processed in spatial chunks of 512 positions:
  conv(1x1) + bias -> PSUM; square + ones-matmul -> ||emb||^2;
  rnorm = exp(-0.5 ln ssq); partition-broadcast rnorm (GPSIMD);
  emb_n = emb * rnorm (DVE); dot products against 1095 prototype rows,
  grouped (ori | neg0 | neg1) into one packed PSUM tile per row-slab so
  the transcendentals run as wide single ACT ops, all from the one ACT
  table set {ln, exp, square}:
     d2 = 2 - 2 dot   (prototypes and emb_n are unit-norm)
     dist = exp(0.5 ln(d2)),  probs_* = exp(affine(...)),
  min/mul/add tail on DVE/GPSIMD, class-sum via ones-matmul, cls = probs/sum.
Outputs are packed per slab ([dist_o|dist_n0|dist_n1] and
[cls_neg|cls|probs_ori]) so each slab needs only two output DMAs.
"""
import sys
sys.path.insert(0, "/opt/trn_rl_repo")

import numpy as np
import concourse.bass as bass
import concourse.tile as tile
from concourse import bacc, mybir
from concourse.bass_utils import run_bass_kernel_spmd

F32 = mybir.dt.float32
F32R = mybir.dt.float32r
AF = mybir.ActivationFunctionType
ALU = mybir.AluOpType

# problem constants (hardcoded per contract)
B, CIN, H, W = 16, 256, 64, 64
E, R, NEG = 256, 365, 2
NCORES = 8
BL = B // NCORES          # batches per core
N = H * W                 # 4096 spatial positions
NCH = 512                 # chunk of positions per inner step
NJ = N // NCH
SIGMA, BETA = 0.5, 0.3
INV2S2 = 1.0 / (2.0 * SIGMA ** 2)   # 2.0

# matmul dtype for conv + dot matmuls: 'f32' (exact, 4 cyc/row) or
# 'f32r' (tf32-like, 1 cyc/row). ssq/sum matmuls stay fp32.
DT_MM_DEFAULT = "f32"

# slabs of prototype rows: (row0, nrows); 365 = 128 + 128 + 109
SLABS = [(0, 128), (128, 128), (256, R - 256)]

LAST_EXEC_TIME_NS = None


def _build(dt_mm: str):
    DTM = F32R if dt_mm == "f32r" else F32
    nc = bacc.Bacc("TRN2", target_bir_lowering=False)

    x_d = nc.dram_tensor("x", [BL, CIN, N], F32, kind="ExternalInput")
    convT_d = nc.dram_tensor("convT", [CIN, E], F32, kind="ExternalInput")
    convb_d = nc.dram_tensor("convb", [1, E], F32, kind="ExternalInput")
    repsT_d = nc.dram_tensor("repsT", [E, 3 * R], F32, kind="ExternalInput")

    # o_pack: (dist_o, dist_n0, dist_n1); o_misc: (cls_neg, cls, probs_ori)
    o_pack = nc.dram_tensor("o_pack", [BL, R, 3, N], F32, kind="ExternalOutput")
    o_misc = nc.dram_tensor("o_misc", [BL, R, 3, N], F32, kind="ExternalOutput")

    with tile.TileContext(nc) as tc:
        with (
            tc.tile_pool(name="const", bufs=1) as const,
            tc.tile_pool(name="io", bufs=3) as io,
            tc.tile_pool(name="mid", bufs=3) as mid,
            tc.tile_pool(name="lnp", bufs=3) as lnp,
            tc.tile_pool(name="dpk", bufs=4) as dpk,
            tc.tile_pool(name="ch", bufs=4) as chp,
            tc.tile_pool(name="ps_emb", bufs=1, space="PSUM") as ps_emb,
            tc.tile_pool(name="ps_dot", bufs=2, space="PSUM") as ps_dot,
        ):
            # ---- resident constants ----
            convT_f = const.tile([128, 2, E], F32)      # [k, ktile, E]
            nc.sync.dma_start(out=convT_f[:],
                              in_=convT_d[:].rearrange("(a k) e -> k a e", k=128))
            convb_f = const.tile([1, E], F32)
            nc.sync.dma_start(out=convb_f[:], in_=convb_d[:])
            repsT_f = const.tile([128, 2, 3 * R], F32)
            nc.sync.dma_start(out=repsT_f[:],
                              in_=repsT_d[:].rearrange("(a k) r -> k a r", k=128))

            ones_f = const.tile([128, 1], F32)
            nc.vector.memset(ones_f[:], 1.0)
            onesr_f = const.tile([1, NCH], F32)
            nc.vector.memset(onesr_f[:], 1.0)
            bias_c = const.tile([128, 1], F32)     # c = en2 + rn2 = 2.0
            nc.vector.memset(bias_c[:], 2.0)
            bias_mc = const.tile([128, 1], F32)    # -INV2S2 * c
            nc.vector.memset(bias_mc[:], -2.0 * INV2S2)

            if DTM is F32R:
                convT_s = const.tile([128, 2, E], F32R)
                nc.vector.tensor_copy(convT_s[:], convT_f[:])
                convb_s = const.tile([1, E], F32R)
                nc.vector.tensor_copy(convb_s[:], convb_f[:])
                repsT_s = const.tile([128, 2, 3 * R], F32R)
                nc.vector.tensor_copy(repsT_s[:], repsT_f[:])
                ones_r = const.tile([1, NCH], F32R)
                nc.vector.tensor_copy(ones_r[:], onesr_f[:])
            else:
                convT_s, convb_s, repsT_s = convT_f, convb_f, repsT_f
                ones_r = onesr_f

            for b in range(BL):
                for j in range(NJ):
                    ns = slice(j * NCH, (j + 1) * NCH)

                    # -- load x chunk (one DMA, 2 k-tiles stacked in free) --
                    xf = io.tile([128, 2, NCH], F32, tag="x")
                    nc.sync.dma_start(
                        out=xf[:], in_=x_d[b].rearrange(
                            "(k a) n -> a k n", a=128)[:, :, ns])
                    if DTM is F32R:
                        xs = io.tile([128, 2, NCH], F32R, tag="xr")
                        nc.gpsimd.tensor_copy(out=xs[:], in_=xf[:])
                    else:
                        xs = xf

                    # -- conv: emb[e, n] in PSUM, e-tiles stacked in free --
                    emb = ps_emb.tile([128, 2, NCH], F32, tag="emb")
                    for m in range(2):
                        mcols = slice(m * 128, (m + 1) * 128)
                        nc.tensor.matmul(emb[:, m, :], convT_s[:, 0, mcols],
                                         xs[:, 0, :], start=True, stop=False)
                        nc.tensor.matmul(emb[:, m, :], convT_s[:, 1, mcols],
                                         xs[:, 1, :], start=False, stop=False)
                        nc.tensor.matmul(emb[:, m, :], convb_s[:, mcols],
                                         ones_r[:], start=False, stop=True)

                    # -- ssq = sum_e emb^2 via square + ones-matmul (fp32) --
                    sq = mid.tile([128, 2, NCH], F32, tag="sq")
                    nc.scalar.activation(sq[:], emb[:], AF.Square)
                    ssq = ps_dot.tile([1, NCH], F32, tag="dot")
                    nc.tensor.matmul(ssq[:], ones_f[:], sq[:, 0, :],
                                     start=True, stop=False)
                    nc.tensor.matmul(ssq[:], ones_f[:], sq[:, 1, :],
                                     start=False, stop=True)

                    # -- rnorm = ssq^-0.5 = exp(-0.5 ln(ssq)) --
                    lnssq = mid.tile([1, NCH], F32, tag="lnssq")
                    nc.scalar.activation(lnssq[:], ssq[:], AF.Ln)
                    rnorm = mid.tile([1, NCH], F32, tag="rnorm")
                    nc.scalar.activation(rnorm[:], lnssq[:], AF.Exp, scale=-0.5)
                    bcast = mid.tile([128, NCH], F32, tag="bcast")
                    nc.gpsimd.partition_broadcast(bcast[:], rnorm[:])

                    # -- emb_n = emb * rnorm (one op, bcast repeated in free) --
                    embn = mid.tile([128, 2, NCH], DTM, tag="embn")
                    bc2 = bass.AP(tensor=bcast.tensor, offset=bcast[:].offset,
                                  ap=[bcast[:].ap[0], [0, 2], bcast[:].ap[1]])
                    nc.vector.tensor_mul(embn[:], emb[:], bc2)

                    # -- dots packed per slab: [p, 3, NCH] = (ori, n0, n1) --
                    for s, (r0, p) in enumerate(SLABS):
                        dp = ps_dot.tile([128, 3, NCH], F32, tag="dot")
                        for g in range(3):
                            col0 = g * R + r0
                            nc.tensor.matmul(
                                dp[:p, g, :], repsT_s[:, 0, col0:col0 + p],
                                embn[:, 0, :], start=True, stop=False)
                            nc.tensor.matmul(
                                dp[:p, g, :], repsT_s[:, 1, col0:col0 + p],
                                embn[:, 1, :], start=False, stop=True)

                        # d2 = 2 - 2 dot (unit-norm emb & prototypes)
                        # ln(d2) over all three groups in one op
                        lnd = lnp.tile([128, 3, NCH], F32, tag="lnd")
                        nc.scalar.activation(lnd[:p, :, :], dp[:p, :, :],
                                             AF.Ln, bias=bias_c[:p, :], scale=-2.0)
                        # dist = exp(0.5 ln d2), packed -> one DMA
                        dist = dpk.tile([128, 3, NCH], F32, tag="dist")
                        nc.scalar.activation(dist[:p, :, :], lnd[:p, :, :],
                                             AF.Exp, scale=0.5)
                        nc.sync.dma_start(out=o_pack[b, r0:r0 + p, :, ns],
                                          in_=dist[:p, :, :])

                        # w slots: 0=probs, 1=cls_neg, 2=cls, 3=probs_ori
                        w = chp.tile([128, 4, NCH], F32, tag="w")
                        nc.scalar.activation(w[:p, 3, :], dp[:p, 0, :],
                                             AF.Exp, bias=bias_mc[:p, :],
                                             scale=2.0 * INV2S2)

                        dnmin = chp.tile([128, NCH], F32, tag="dnmin")
                        nc.vector.tensor_tensor(
                            dnmin[:p, :], dist[:p, 1, :], dist[:p, 2, :],
                            op=ALU.min)
                        # u slots: 0=t2, 1=dnmin^2
                        u = chp.tile([128, 2, NCH], F32, tag="u")
                        nc.gpsimd.tensor_mul(u[:p, 1, :], dnmin[:p, :],
                                             dnmin[:p, :])
                        st = chp.tile([128, NCH], F32, tag="st")
                        nc.gpsimd.tensor_scalar(
                            out=st[:p, :], in0=dnmin[:p, :],
                            scalar1=2.0, scalar2=-BETA,
                            op0=ALU.subtract, op1=ALU.mult)
                        t = chp.tile([128, NCH], F32, tag="t")
                        nc.vector.tensor_add(t[:p, :], dist[:p, 0, :], st[:p, :])
                        nc.vector.tensor_mul(u[:p, 0, :], t[:p, :], t[:p, :])
                        # (probs, cls_neg) = exp(-INV2S2 * (t2, dnmin2))
                        nc.scalar.activation(w[:p, 0:2, :], u[:p, :, :],
                                             AF.Exp, scale=-INV2S2)

                        if s == 0:
                            psum = ps_dot.tile([1, NCH], F32, tag="dot")
                        nc.tensor.matmul(psum[:], ones_f[:p, :], w[:p, 0, :],
                                         start=(s == 0), stop=(s == 2))
                        if s == 0:
                            w0, p0 = w, p
                        elif s == 1:
                            w1, p1 = w, p
                        else:
                            w2, p2 = w, p

                    rsum = mid.tile([1, NCH], F32, tag="rsum")
                    nc.vector.reciprocal(rsum[:], psum[:])
                    rbc = mid.tile([128, NCH], F32, tag="rbc")
                    nc.gpsimd.partition_broadcast(rbc[:], rsum[:])
                    for s, (r0, p) in enumerate(SLABS):
                        w = (w0, w1, w2)[s]
                        nc.vector.tensor_mul(w[:p, 2, :], w[:p, 0, :],
                                             rbc[:p, :])
                        nc.sync.dma_start(out=o_misc[b, r0:r0 + p, :, ns],
                                          in_=w[:p, 1:4, :])
    nc.compile()
    return nc


_NC_CACHE = {}


def _host_prep(x, conv_w, conv_b, representations, neg_w, neg_b):
    f = np.float32
    x = np.asarray(x, f)
    conv_w = np.asarray(conv_w, f)
    conv_b = np.asarray(conv_b, f)
    reps = np.asarray(representations, f)
    neg_w = np.asarray(neg_w, f)
    neg_b = np.asarray(neg_b, f)

    r0 = reps[:, 0, :]                                     # [R, E]
    off = (np.abs(r0) @ neg_w.T + neg_b).reshape(R, NEG, E).astype(f)
    rneg = ((off + np.abs(reps)) * np.sign(reps)).astype(f)
    nrm = np.sqrt((rneg * rneg).sum(2, keepdims=True, dtype=f))
    rneg = (rneg / np.maximum(nrm, 1e-12)).astype(f)

    # repsT: [E, 3R] columns = [ori | neg m0 | neg m1]
    allr = np.concatenate([r0[None], rneg[:, 0][None], rneg[:, 1][None]], 0)
    repsT = np.ascontiguousarray(allr.reshape(3 * R, E).T).astype(f)

    convT = np.ascontiguousarray(conv_w.T).astype(f)       # [CIN, E]
    convb2 = conv_b.reshape(1, E).astype(f)

    shared = {"convT": convT, "convb": convb2, "repsT": repsT}
    in_maps = []
    for i in range(NCORES):
        m = dict(shared)
        m["x"] = np.ascontiguousarray(
            x[i * BL:(i + 1) * BL].reshape(BL, CIN, N))
        in_maps.append(m)
    return in_maps


def _run(inputs, dt_mm=DT_MM_DEFAULT, trace=False):
    global LAST_EXEC_TIME_NS
    in_maps = _host_prep(**inputs)
    if dt_mm not in _NC_CACHE:
        _NC_CACHE[dt_mm] = _build(dt_mm)
    nc = _NC_CACHE[dt_mm]
    res = run_bass_kernel_spmd(nc, in_maps, list(range(NCORES)), trace=trace)
    LAST_EXEC_TIME_NS = res.exec_time_ns

    def cat(name):
        return np.concatenate([res.results[i][name] for i in range(NCORES)], 0)

    pack = cat("o_pack").reshape(B, R, 3, H, W)
    misc = cat("o_misc").reshape(B, R, 3, H, W)
    distance = np.ascontiguousarray(pack[:, :, 0:1])
    distance_neg = np.ascontiguousarray(pack[:, :, 1:3])
    cls_neg = np.ascontiguousarray(misc[:, :, 0])
    cls_score = np.ascontiguousarray(misc[:, :, 1])
    probs_ori = np.ascontiguousarray(misc[:, :, 2])
    return cls_score, cls_neg, distance, distance_neg, probs_ori


def kernel(**inputs):
    return _run(inputs, trace=False)


if __name__ == "__main__":
    print("kernel module; use test.py")


# revision 8
# speedup vs baseline: 1.0488x; 1.0488x over previous
"""Trainium2 Bass kernel for nn_DMLNegHead (retrieval_knn head).

Strategy: data-parallel over batch B=16 across 8 NeuronCores (2 images per
core), prototype/offset params replicated. No collectives needed — every
output has a leading batch axis.

Per-core pipeline, processed in spatial chunks of 512 positions:
  conv(1x1) -> PSUM; (emb+bias)^2 on ACT + ones-matmul -> ||emb||^2;
  rnorm = exp(-0.5 ln ssq); partition-broadcast rnorm (GPSIMD);
  emb_n = (emb+bias) * rnorm (DVE scalar_tensor_tensor);
  dot products against 1095 prototype rows, grouped (ori|neg0|neg1) into one
  packed PSUM tile per row-slab so transcendentals run as wide single ACT
  ops, all funcs pinned to the one ACT table set {ln, exp, square}:
     d2 = 2 - 2 dot   (prototypes and emb_n are unit-norm)
     dist = exp(0.5 ln(d2)),  probs_* = exp(affine(...))
  min/mul/add tail on DVE/GPSIMD, class-sum via ones-matmul,
  cls = probs * exp(-ln(sum)).
Outputs are packed ([dist_o|dist_n0|dist_n1], [cls_neg|cls|probs_ori]) and
DMA'd once per two chunks (4 KB contiguous rows) to halve descriptor work.
"""
import sys
sys.path.insert(0, "/opt/trn_rl_repo")

import numpy as np
import concourse.bass as bass
import concourse.tile as tile
from concourse import bacc, mybir, hw_specs
from concourse.bass_utils import run_bass_kernel_spmd

F32 = mybir.dt.float32
F32R = mybir.dt.float32r
AF = mybir.ActivationFunctionType
ALU = mybir.AluOpType

# problem constants (hardcoded per contract)
B, CIN, H, W = 16, 256, 64, 64
E, R, NEG = 256, 365, 2
NCORES = 8
BL = B // NCORES          # batches per core
N = H * W                 # 4096 spatial positions
NCH = 512                 # chunk of positions per inner step
NJ = N // NCH
SIGMA, BETA = 0.5, 0.3
INV2S2 = 1.0 / (2.0 * SIGMA ** 2)   # 2.0

# matmul dtype for conv + dot matmuls: 'f32' (exact, 4 cyc/row) or
# 'f32r' (tf32-like, 1 cyc/row). ssq/sum matmuls stay fp32.
DT_MM_DEFAULT = "f32"

# slabs of prototype rows: (row0, nrows); 365 = 128 + 128 + 109
SLABS = [(0, 128), (128, 128), (256, R - 256)]

LAST_EXEC_TIME_NS = None

_ACT_SET = "natural_log_exp_and_others"
_ACT_PINNED = False


def _pin_act_tables():
    """Make natural_log_exp_and_others the only candidate set for the
    functions this kernel uses, so the table-load pass emits one load
    instead of ping-ponging between per-function default sets."""
    global _ACT_PINNED
    if _ACT_PINNED:
        return
    tabs = hw_specs.get_activation_tables("gen3")
    pinned = {AF.Exp, AF.Ln, AF.Square}
    assert pinned <= tabs[_ACT_SET]
    for name, funcs in tabs.items():
        if name != _ACT_SET:
            funcs -= pinned
    _ACT_PINNED = True


def _build(dt_mm: str):
    DTM = F32R if dt_mm == "f32r" else F32
    _pin_act_tables()
    nc = bacc.Bacc("TRN2", target_bir_lowering=False)

    x_d = nc.dram_tensor("x", [BL, CIN, N], F32, kind="ExternalInput")
    convT_d = nc.dram_tensor("convT", [CIN, E], F32, kind="ExternalInput")
    convb_d = nc.dram_tensor("convb", [128, 2], F32, kind="ExternalInput")
    repsT_d = nc.dram_tensor("repsT", [E, 3 * R], F32, kind="ExternalInput")

    # o_pack: (dist_o, dist_n0, dist_n1); o_misc: (cls_neg, cls, probs_ori)
    o_pack = nc.dram_tensor("o_pack", [BL, R, 3, N], F32, kind="ExternalOutput")
    o_misc = nc.dram_tensor("o_misc", [BL, R, 3, N], F32, kind="ExternalOutput")

    with tile.TileContext(nc) as tc:
        with (
            tc.tile_pool(name="const", bufs=1) as const,
            tc.tile_pool(name="io", bufs=2) as io,
            tc.tile_pool(name="mid", bufs=2) as mid,
            tc.tile_pool(name="lnp", bufs=2) as lnp,
            tc.tile_pool(name="dpk", bufs=4) as dpk,
            tc.tile_pool(name="ch", bufs=2) as chp,
            tc.tile_pool(name="wp", bufs=3) as wp,
            tc.tile_pool(name="ps_emb", bufs=2, space="PSUM") as ps_emb,
            tc.tile_pool(name="ps_dot", bufs=1, space="PSUM") as ps_dot,
            tc.tile_pool(name="ps_ssq", bufs=1, space="PSUM") as ps_ssq,
        ):
            # ---- resident constants ----
            convT_f = const.tile([128, 2, E], F32)      # [k, ktile, E]
            nc.sync.dma_start(out=convT_f[:],
                              in_=convT_d[:].rearrange("(a k) e -> k a e", k=128))
            convb = const.tile([128, 2], F32)           # per-partition bias, e-tile
            nc.sync.dma_start(out=convb[:], in_=convb_d[:])
            repsT_f = const.tile([128, 2, 3 * R], F32)
            nc.sync.dma_start(out=repsT_f[:],
                              in_=repsT_d[:].rearrange("(a k) r -> k a r", k=128))

            ones_f = const.tile([128, 1], F32)
            nc.vector.memset(ones_f[:], 1.0)
            bias_c = const.tile([128, 1], F32)     # c = en2 + rn2 = 2.0
            nc.vector.memset(bias_c[:], 2.0)
            bias_mc = const.tile([128, 1], F32)    # -INV2S2 * c
            nc.vector.memset(bias_mc[:], -2.0 * INV2S2)

            if DTM is F32R:
                convT_s = const.tile([128, 2, E], F32R)
                nc.vector.tensor_copy(convT_s[:], convT_f[:])
                repsT_s = const.tile([128, 2, 3 * R], F32R)
                nc.vector.tensor_copy(repsT_s[:], repsT_f[:])
            else:
                convT_s, repsT_s = convT_f, repsT_f

            for b in range(BL):
                xs_pair = None
                dist_pair = [None] * 3
                w_pair = [None] * 3
                for j in range(NJ):
                    jj = j % 2
                    ns = slice(j * NCH, (j + 1) * NCH)

                    # -- load x chunk --
                    xf = io.tile([128, 2, NCH], F32, tag="x")
                    nc.sync.dma_start(
                        out=xf[:], in_=x_d[b].rearrange(
                            "(k a) n -> a k n", a=128)[:, :, ns])
                    if DTM is F32R:
                        xs = io.tile([128, 2, NCH], F32R, tag="xr")
                        nc.gpsimd.tensor_copy(out=xs[:], in_=xf[:])
                    else:
                        xs = xf

                    # -- conv: emb[e, n] in PSUM, e-tiles stacked in free --
                    emb = ps_emb.tile([128, 2, NCH], F32, tag="emb")
                    for m in range(2):
                        mcols = slice(m * 128, (m + 1) * 128)
                        nc.tensor.matmul(emb[:, m, :], convT_s[:, 0, mcols],
                                         xs[:, 0, :], start=True, stop=False)
                        nc.tensor.matmul(emb[:, m, :], convT_s[:, 1, mcols],
                                         xs[:, 1, :], start=False, stop=True)

                    # -- ssq = sum_e (emb+b)^2 via Square(bias) + ones-mm --
                    sq = mid.tile([128, 2, NCH], F32, tag="sq")
                    for m in range(2):
                        nc.scalar.activation(sq[:, m, :], emb[:, m, :],
                                             AF.Square, bias=convb[:, m:m + 1])
                    ssq = ps_ssq.tile([1, NCH], F32, tag="ssq")
                    nc.tensor.matmul(ssq[:], ones_f[:], sq[:, 0, :],
                                     start=True, stop=False)
                    nc.tensor.matmul(ssq[:], ones_f[:], sq[:, 1, :],
                                     start=False, stop=True)

                    # -- rnorm = ssq^-0.5 = exp(-0.5 ln(ssq)) --
                    lnssq = mid.tile([1, NCH], F32, tag="lnssq")
                    nc.scalar.activation(lnssq[:], ssq[:], AF.Ln)
                    rnorm = mid.tile([1, NCH], F32, tag="rnorm")
                    nc.scalar.activation(rnorm[:], lnssq[:], AF.Exp, scale=-0.5)
                    bcast = mid.tile([128, NCH], F32, tag="bcast")
                    nc.gpsimd.partition_broadcast(bcast[:], rnorm[:])

                    # -- emb_n = (emb + b) * rnorm --
                    embn = mid.tile([128, 2, NCH], DTM, tag="embn")
                    for m in range(2):
                        nc.vector.scalar_tensor_tensor(
                            out=embn[:, m, :], in0=emb[:, m, :],
                            scalar=convb[:, m:m + 1], in1=bcast[:],
                            op0=ALU.add, op1=ALU.mult)

                    # -- dots packed per slab: [p, 3, NCH] = (ori, n0, n1) --
                    for s, (r0, p) in enumerate(SLABS):
                        dp = ps_dot.tile([128, 3, NCH], F32, tag="dot")
                        for g in range(3):
                            col0 = g * R + r0
                            nc.tensor.matmul(
                                dp[:p, g, :], repsT_s[:, 0, col0:col0 + p],
                                embn[:, 0, :], start=True, stop=False)
                            nc.tensor.matmul(
                                dp[:p, g, :], repsT_s[:, 1, col0:col0 + p],
                                embn[:, 1, :], start=False, stop=True)

                        # d2 = 2 - 2 dot; ln over all three groups in one op
                        lnd = lnp.tile([128, 3, NCH], F32, tag="lnd")
                        nc.scalar.activation(lnd[:p, :, :], dp[:p, :, :],
                                             AF.Ln, bias=bias_c[:p, :],
                                             scale=-2.0)
                        # dist = exp(0.5 ln d2) into the pair-accumulated tile
                        if jj == 0:
                            dist_pair[s] = dpk.tile([128, 3, 2, NCH], F32,
                                                    tag="dist", name="dist")
                            w_pair[s] = wp.tile([128, 3, 2, NCH], F32,
                                                tag="w", name="w")
                        dist = dist_pair[s]
                        w = w_pair[s]
                        nc.scalar.activation(dist[:p, :, jj, :], lnd[:p, :, :],
                                             AF.Exp, scale=0.5)
                        # probs_ori = exp(4 dot - 4) from the ori dot slice
                        nc.scalar.activation(w[:p, 2, jj, :], dp[:p, 0, :],
                                             AF.Exp, bias=bias_mc[:p, :],
                                             scale=2.0 * INV2S2)

                        dnmin = chp.tile([128, NCH], F32, tag="dnmin")
                        nc.vector.tensor_tensor(
                            dnmin[:p, :], dist[:p, 1, jj, :],
                            dist[:p, 2, jj, :], op=ALU.min)
                        # u slots: 0=dnmin^2, 1=t^2
                        u = chp.tile([128, 2, NCH], F32, tag="u")
                        nc.gpsimd.tensor_mul(u[:p, 0, :], dnmin[:p, :],
                                             dnmin[:p, :])
                        st = chp.tile([128, NCH], F32, tag="st")
                        nc.vector.tensor_scalar(
                            out=st[:p, :], in0=dnmin[:p, :],
                            scalar1=2.0, scalar2=-BETA,
                            op0=ALU.subtract, op1=ALU.mult)
                        t = chp.tile([128, NCH], F32, tag="t")
                        nc.vector.tensor_add(t[:p, :], dist[:p, 0, jj, :],
                                             st[:p, :])
                        nc.vector.tensor_mul(u[:p, 1, :], t[:p, :], t[:p, :])
                        # (cls_neg, probs) = exp(-INV2S2 * (dnmin2, t2))
                        nc.scalar.activation(w[:p, 0:2, jj, :], u[:p, :, :],
                                             AF.Exp, scale=-INV2S2)

                        if s == 0:
                            psum = ps_emb.tile([1, NCH], F32, tag="emb")
                        nc.tensor.matmul(psum[:], ones_f[:p, :],
                                         w[:p, 1, jj, :],
                                         start=(s == 0), stop=(s == 2))

                    # cls = probs * exp(-ln(sum))
                    lnsum = mid.tile([1, NCH], F32, tag="lnsum")
                    nc.scalar.activation(lnsum[:], psum[:], AF.Ln)
                    rsum = mid.tile([1, NCH], F32, tag="rsum")
                    nc.scalar.activation(rsum[:], lnsum[:], AF.Exp, scale=-1.0)
                    rbc = mid.tile([128, NCH], F32, tag="rbc")
                    nc.gpsimd.partition_broadcast(rbc[:], rsum[:])
                    for s, (r0, p) in enumerate(SLABS):
                        w = w_pair[s]
                        nc.vector.tensor_mul(w[:p, 1, jj, :], w[:p, 1, jj, :],
                                             rbc[:p, :])

                    # -- pair-wide output DMAs (4 KB rows) --
                    if jj == 1:
                        n2 = slice((j - 1) * NCH, (j + 1) * NCH)
                        for s, (r0, p) in enumerate(SLABS):
                            nc.sync.dma_start(
                                out=o_pack[b, r0:r0 + p, :, n2],
                                in_=dist_pair[s][:p, :, :, :])
                            nc.sync.dma_start(
                                out=o_misc[b, r0:r0 + p, :, n2],
                                in_=w_pair[s][:p, :, :, :])
    nc.compile()
    return nc


_NC_CACHE = {}


def _host_prep(x, conv_w, conv_b, representations, neg_w, neg_b):
    f = np.float32
    x = np.asarray(x, f)
    conv_w = np.asarray(conv_w, f)
    conv_b = np.asarray(conv_b, f)
    reps = np.asarray(representations, f)
    neg_w = np.asarray(neg_w, f)
    neg_b = np.asarray(neg_b, f)

    r0 = reps[:, 0, :]                                     # [R, E]
    off = (np.abs(r0) @ neg_w.T + neg_b).reshape(R, NEG, E).astype(f)
    rneg = ((off + np.abs(reps)) * np.sign(reps)).astype(f)
    nrm = np.sqrt((rneg * rneg).sum(2, keepdims=True, dtype=f))
    rneg = (rneg / np.maximum(nrm, 1e-12)).astype(f)

    # repsT: [E, 3R] columns = [ori | neg m0 | neg m1]
    allr = np.concatenate([r0[None], rneg[:, 0][None], rneg[:, 1][None]], 0)
    repsT = np.ascontiguousarray(allr.reshape(3 * R, E).T).astype(f)

    convT = np.ascontiguousarray(conv_w.T).astype(f)       # [CIN, E]
    convb2 = np.ascontiguousarray(conv_b.reshape(2, 128).T)  # [128, ktile]

    shared = {"convT": convT, "convb": convb2, "repsT": repsT}
    in_maps = []
    for i in range(NCORES):
        m = dict(shared)
        m["x"] = np.ascontiguousarray(
            x[i * BL:(i + 1) * BL].reshape(BL, CIN, N))
        in_maps.append(m)
    return in_maps


def _run(inputs, dt_mm=DT_MM_DEFAULT, trace=False):
    global LAST_EXEC_TIME_NS
    in_maps = _host_prep(**inputs)
    if dt_mm not in _NC_CACHE:
        _NC_CACHE[dt_mm] = _build(dt_mm)
    nc = _NC_CACHE[dt_mm]
    res = run_bass_kernel_spmd(nc, in_maps, list(range(NCORES)), trace=trace)
    LAST_EXEC_TIME_NS = res.exec_time_ns

    def cat(name):
        return np.concatenate([res.results[i][name] for i in range(NCORES)], 0)

    pack = cat("o_pack").reshape(B, R, 3, H, W)
    misc = cat("o_misc").reshape(B, R, 3, H, W)
    distance = np.ascontiguousarray(pack[:, :, 0:1])
    distance_neg = np.ascontiguousarray(pack[:, :, 1:3])
    cls_neg = np.ascontiguousarray(misc[:, :, 0])
    cls_score = np.ascontiguousarray(misc[:, :, 1])
    probs_ori = np.ascontiguousarray(misc[:, :, 2])
    return cls_score, cls_neg, distance, distance_neg, probs_ori


def kernel(**inputs):
    return _run(inputs, trace=False)


if __name__ == "__main__":
    print("kernel module; use test.py")


# revision 9
# speedup vs baseline: 1.1858x; 1.1306x over previous
"""Trainium2 Bass kernel for nn_DMLNegHead (retrieval_knn head).

Strategy: data-parallel over batch B=16 across 8 NeuronCores (2 images per
core), prototype/offset params replicated. No collectives needed — every
output has a leading batch axis.

Per-core pipeline, processed in spatial chunks of 512 positions:
  conv(1x1) -> PSUM; (emb+bias)^2 on ACT + ones-matmul -> ||emb||^2;
  rnorm = exp(-0.5 ln ssq); partition-broadcast rnorm (GPSIMD);
  emb_n = (emb+bias) * rnorm (DVE scalar_tensor_tensor);
  dot products against 1095 prototype rows, grouped (ori|neg0|neg1) into one
  packed PSUM tile per row-slab so transcendentals run as wide single ACT
  ops, all funcs pinned to the one ACT table set {ln, exp, square}:
     d2 = 2 - 2 dot   (prototypes and emb_n are unit-norm)
     dist = exp(0.5 ln(d2)),  probs_* = exp(affine(...))
  min/mul/add tail on DVE/GPSIMD, class-sum via ones-matmul,
  cls = probs * exp(-ln(sum)).
Outputs are packed ([dist_o|dist_n0|dist_n1], [cls_neg|cls|probs_ori]) and
DMA'd once per two chunks (4 KB contiguous rows) to halve descriptor work.
"""
import sys
sys.path.insert(0, "/opt/trn_rl_repo")

import numpy as np
import concourse.bass as bass
import concourse.tile as tile
from concourse import bacc, mybir, hw_specs
from concourse.bass_utils import run_bass_kernel_spmd

F32 = mybir.dt.float32
F32R = mybir.dt.float32r
AF = mybir.ActivationFunctionType
ALU = mybir.AluOpType

# problem constants (hardcoded per contract)
B, CIN, H, W = 16, 256, 64, 64
E, R, NEG = 256, 365, 2
NCORES = 8
BL = B // NCORES          # batches per core
N = H * W                 # 4096 spatial positions
NCH = 512                 # chunk of positions per inner step
NJ = N // NCH
SIGMA, BETA = 0.5, 0.3
INV2S2 = 1.0 / (2.0 * SIGMA ** 2)   # 2.0

# matmul dtype for conv + dot matmuls: 'f32' (exact, 4 cyc/row) or
# 'f32r' (tf32-like, 1 cyc/row). ssq/sum matmuls stay fp32.
DT_MM_DEFAULT = "f32r"

# slabs of prototype rows: (row0, nrows); 365 = 128 + 128 + 109
SLABS = [(0, 128), (128, 128), (256, R - 256)]

LAST_EXEC_TIME_NS = None

_ACT_SET = "natural_log_exp_and_others"
_ACT_PINNED = False


def _pin_act_tables():
    """Make natural_log_exp_and_others the only candidate set for the
    functions this kernel uses, so the table-load pass emits one load
    instead of ping-ponging between per-function default sets."""
    global _ACT_PINNED
    if _ACT_PINNED:
        return
    tabs = hw_specs.get_activation_tables("gen3")
    pinned = {AF.Exp, AF.Ln, AF.Square}
    assert pinned <= tabs[_ACT_SET]
    for name, funcs in tabs.items():
        if name != _ACT_SET:
            funcs -= pinned
    _ACT_PINNED = True


def _build(dt_mm: str):
    DTM = F32R if dt_mm == "f32r" else F32
    _pin_act_tables()
    nc = bacc.Bacc("TRN2", target_bir_lowering=False)

    x_d = nc.dram_tensor("x", [BL, CIN, N], F32, kind="ExternalInput")
    convT_d = nc.dram_tensor("convT", [CIN, E], F32, kind="ExternalInput")
    convb_d = nc.dram_tensor("convb", [128, 2], F32, kind="ExternalInput")
    repsT_d = nc.dram_tensor("repsT", [E, 3 * R], F32, kind="ExternalInput")

    # o_pack: (dist_o, dist_n0, dist_n1); o_misc: (cls_neg, cls, probs_ori)
    o_pack = nc.dram_tensor("o_pack", [BL, R, 3, N], F32, kind="ExternalOutput")
    o_misc = nc.dram_tensor("o_misc", [BL, R, 3, N], F32, kind="ExternalOutput")

    with tile.TileContext(nc) as tc:
        with (
            tc.tile_pool(name="const", bufs=1) as const,
            tc.tile_pool(name="io", bufs=2) as io,
            tc.tile_pool(name="mid", bufs=2) as mid,
            tc.tile_pool(name="lnp", bufs=2) as lnp,
            tc.tile_pool(name="dpk", bufs=4) as dpk,
            tc.tile_pool(name="ch", bufs=2) as chp,
            tc.tile_pool(name="wp", bufs=3) as wp,
            tc.tile_pool(name="ps_emb", bufs=3, space="PSUM") as ps_emb,
            tc.tile_pool(name="ps_dot", bufs=3, space="PSUM") as ps_dot,
            tc.tile_pool(name="ps_ssq", bufs=2, space="PSUM") as ps_ssq,
        ):
            # ---- resident constants ----
            convT_f = const.tile([128, 2, E], F32)      # [k, ktile, E]
            nc.sync.dma_start(out=convT_f[:],
                              in_=convT_d[:].rearrange("(a k) e -> k a e", k=128))
            convb = const.tile([128, 2], F32)           # per-partition bias, e-tile
            nc.sync.dma_start(out=convb[:], in_=convb_d[:])
            repsT_f = const.tile([128, 2, 3 * R], F32)
            nc.sync.dma_start(out=repsT_f[:],
                              in_=repsT_d[:].rearrange("(a k) r -> k a r", k=128))

            ones_f = const.tile([128, 1], F32)
            nc.vector.memset(ones_f[:], 1.0)
            bias_c = const.tile([128, 1], F32)     # c = en2 + rn2 = 2.0
            nc.vector.memset(bias_c[:], 2.0)
            bias_mc = const.tile([128, 1], F32)    # -INV2S2 * c
            nc.vector.memset(bias_mc[:], -2.0 * INV2S2)

            if DTM is F32R:
                convT_s = const.tile([128, 2, E], F32R)
                nc.vector.tensor_copy(convT_s[:], convT_f[:])
                repsT_s = const.tile([128, 2, 3 * R], F32R)
                nc.vector.tensor_copy(repsT_s[:], repsT_f[:])
            else:
                convT_s, repsT_s = convT_f, repsT_f

            for b in range(BL):
                xs_pair = None
                dist_pair = [None] * 3
                w_pair = [None] * 3
                for j in range(NJ):
                    jj = j % 2
                    ns = slice(j * NCH, (j + 1) * NCH)

                    # -- load x chunk --
                    xf = io.tile([128, 2, NCH], F32, tag="x")
                    nc.sync.dma_start(
                        out=xf[:], in_=x_d[b].rearrange(
                            "(k a) n -> a k n", a=128)[:, :, ns])
                    if DTM is F32R:
                        xs = io.tile([128, 2, NCH], F32R, tag="xr")
                        nc.gpsimd.tensor_copy(out=xs[:], in_=xf[:])
                    else:
                        xs = xf

                    # -- conv: emb[e, n] in PSUM (two 1-bank e-tiles) --
                    embp = []
                    sq = mid.tile([128, 2, NCH], F32, tag="sq")
                    for m in range(2):
                        em = ps_emb.tile([128, NCH], F32, tag="emb", name="emb")
                        mcols = slice(m * 128, (m + 1) * 128)
                        nc.tensor.matmul(em[:], convT_s[:, 0, mcols],
                                         xs[:, 0, :], start=True, stop=False)
                        nc.tensor.matmul(em[:], convT_s[:, 1, mcols],
                                         xs[:, 1, :], start=False, stop=True)
                        embp.append(em)
                        # ssq operand: (emb+b)^2
                        nc.scalar.activation(sq[:, m, :], em[:],
                                             AF.Square, bias=convb[:, m:m + 1])
                    ssq = ps_ssq.tile([1, NCH], F32, tag="ssq")
                    nc.tensor.matmul(ssq[:], ones_f[:], sq[:, 0, :],
                                     start=True, stop=False)
                    nc.tensor.matmul(ssq[:], ones_f[:], sq[:, 1, :],
                                     start=False, stop=True)

                    # -- rnorm = ssq^-0.5 = exp(-0.5 ln(ssq)) --
                    lnssq = mid.tile([1, NCH], F32, tag="lnssq")
                    nc.scalar.activation(lnssq[:], ssq[:], AF.Ln)
                    rnorm = mid.tile([1, NCH], F32, tag="rnorm")
                    nc.scalar.activation(rnorm[:], lnssq[:], AF.Exp, scale=-0.5)
                    bcast = mid.tile([128, NCH], F32, tag="bcast")
                    nc.gpsimd.partition_broadcast(bcast[:], rnorm[:])

                    # -- emb_n = (emb + b) * rnorm --
                    embn = mid.tile([128, 2, NCH], DTM, tag="embn")
                    for m in range(2):
                        nc.vector.scalar_tensor_tensor(
                            out=embn[:, m, :], in0=embp[m][:],
                            scalar=convb[:, m:m + 1], in1=bcast[:],
                            op0=ALU.add, op1=ALU.mult)

                    # -- dots per (slab, group) in 1-bank PSUM tiles --
                    for s, (r0, p) in enumerate(SLABS):
                        lnd = lnp.tile([128, 3, NCH], F32, tag="lnd")
                        for g in range(3):
                            col0 = g * R + r0
                            dp = ps_dot.tile([128, NCH], F32, tag="dot",
                                             name="dot")
                            nc.tensor.matmul(
                                dp[:p, :], repsT_s[:, 0, col0:col0 + p],
                                embn[:, 0, :], start=True, stop=False)
                            nc.tensor.matmul(
                                dp[:p, :], repsT_s[:, 1, col0:col0 + p],
                                embn[:, 1, :], start=False, stop=True)
                            # d2 = 2 - 2 dot
                            nc.scalar.activation(lnd[:p, g, :], dp[:p, :],
                                                 AF.Ln, bias=bias_c[:p, :],
                                                 scale=-2.0)
                            if g == 0:
                                dp0 = dp
                        # dist = exp(0.5 ln d2) into the pair-accumulated tile
                        if jj == 0:
                            dist_pair[s] = dpk.tile([128, 3, 2, NCH], F32,
                                                    tag="dist", name="dist")
                            w_pair[s] = wp.tile([128, 3, 2, NCH], F32,
                                                tag="w", name="w")
                        dist = dist_pair[s]
                        w = w_pair[s]
                        nc.scalar.activation(dist[:p, :, jj, :], lnd[:p, :, :],
                                             AF.Exp, scale=0.5)
                        # probs_ori = exp(4 dot - 4) from the ori dot tile
                        nc.scalar.activation(w[:p, 2, jj, :], dp0[:p, :],
                                             AF.Exp, bias=bias_mc[:p, :],
                                             scale=2.0 * INV2S2)

                        dnmin = chp.tile([128, NCH], F32, tag="dnmin")
                        nc.vector.tensor_tensor(
                            dnmin[:p, :], dist[:p, 1, jj, :],
                            dist[:p, 2, jj, :], op=ALU.min)
                        # u slots: 0=dnmin^2, 1=t^2
                        u = chp.tile([128, 2, NCH], F32, tag="u")
                        nc.gpsimd.tensor_mul(u[:p, 0, :], dnmin[:p, :],
                                             dnmin[:p, :])
                        st = chp.tile([128, NCH], F32, tag="st")
                        nc.vector.tensor_scalar(
                            out=st[:p, :], in0=dnmin[:p, :],
                            scalar1=2.0, scalar2=-BETA,
                            op0=ALU.subtract, op1=ALU.mult)
                        t = chp.tile([128, NCH], F32, tag="t")
                        nc.vector.tensor_add(t[:p, :], dist[:p, 0, jj, :],
                                             st[:p, :])
                        nc.vector.tensor_mul(u[:p, 1, :], t[:p, :], t[:p, :])
                        # (cls_neg, probs) = exp(-INV2S2 * (dnmin2, t2))
                        nc.scalar.activation(w[:p, 0:2, jj, :], u[:p, :, :],
                                             AF.Exp, scale=-INV2S2)

                        if s == 0:
                            psum = ps_ssq.tile([1, NCH], F32, tag="ssq",
                                               name="psum")
                        nc.tensor.matmul(psum[:], ones_f[:p, :],
                                         w[:p, 1, jj, :],
                                         start=(s == 0), stop=(s == 2))

                    # cls = probs * exp(-ln(sum))
                    lnsum = mid.tile([1, NCH], F32, tag="lnsum")
                    nc.scalar.activation(lnsum[:], psum[:], AF.Ln)
                    rsum = mid.tile([1, NCH], F32, tag="rsum")
                    nc.scalar.activation(rsum[:], lnsum[:], AF.Exp, scale=-1.0)
                    rbc = mid.tile([128, NCH], F32, tag="rbc")
                    nc.gpsimd.partition_broadcast(rbc[:], rsum[:])
                    for s, (r0, p) in enumerate(SLABS):
                        w = w_pair[s]
                        nc.vector.tensor_mul(w[:p, 1, jj, :], w[:p, 1, jj, :],
                                             rbc[:p, :])

                    # -- pair-wide output DMAs (4 KB rows) --
                    if jj == 1:
                        n2 = slice((j - 1) * NCH, (j + 1) * NCH)
                        for s, (r0, p) in enumerate(SLABS):
                            nc.sync.dma_start(
                                out=o_pack[b, r0:r0 + p, :, n2],
                                in_=dist_pair[s][:p, :, :, :])
                            nc.scalar.dma_start(
                                out=o_misc[b, r0:r0 + p, :, n2],
                                in_=w_pair[s][:p, :, :, :])
    nc.compile()
    return nc


_NC_CACHE = {}


def _host_prep(x, conv_w, conv_b, representations, neg_w, neg_b):
    f = np.float32
    x = np.asarray(x, f)
    conv_w = np.asarray(conv_w, f)
    conv_b = np.asarray(conv_b, f)
    reps = np.asarray(representations, f)
    neg_w = np.asarray(neg_w, f)
    neg_b = np.asarray(neg_b, f)

    r0 = reps[:, 0, :]                                     # [R, E]
    off = (np.abs(r0) @ neg_w.T + neg_b).reshape(R, NEG, E).astype(f)
    rneg = ((off + np.abs(reps)) * np.sign(reps)).astype(f)
    nrm = np.sqrt((rneg * rneg).sum(2, keepdims=True, dtype=f))
    rneg = (rneg / np.maximum(nrm, 1e-12)).astype(f)

    # repsT: [E, 3R] columns = [ori | neg m0 | neg m1]
    allr = np.concatenate([r0[None], rneg[:, 0][None], rneg[:, 1][None]], 0)
    repsT = np.ascontiguousarray(allr.reshape(3 * R, E).T).astype(f)

    convT = np.ascontiguousarray(conv_w.T).astype(f)       # [CIN, E]
    convb2 = np.ascontiguousarray(conv_b.reshape(2, 128).T)  # [128, ktile]

    shared = {"convT": convT, "convb": convb2, "repsT": repsT}
    in_maps = []
    for i in range(NCORES):
        m = dict(shared)
        m["x"] = np.ascontiguousarray(
            x[i * BL:(i + 1) * BL].reshape(BL, CIN, N))
        in_maps.append(m)
    return in_maps


def _run(inputs, dt_mm=DT_MM_DEFAULT, trace=False):
    global LAST_EXEC_TIME_NS
    in_maps = _host_prep(**inputs)
    if dt_mm not in _NC_CACHE:
        _NC_CACHE[dt_mm] = _build(dt_mm)
    nc = _NC_CACHE[dt_mm]
    res = run_bass_kernel_spmd(nc, in_maps, list(range(NCORES)), trace=trace)
    LAST_EXEC_TIME_NS = res.exec_time_ns

    def cat(name):
        return np.concatenate([res.results[i][name] for i in range(NCORES)], 0)

    pack = cat("o_pack").reshape(B, R, 3, H, W)
    misc = cat("o_misc").reshape(B, R, 3, H, W)
    distance = np.ascontiguousarray(pack[:, :, 0:1])
    distance_neg = np.ascontiguousarray(pack[:, :, 1:3])
    cls_neg = np.ascontiguousarray(misc[:, :, 0])
    cls_score = np.ascontiguousarray(misc[:, :, 1])
    probs_ori = np.ascontiguousarray(misc[:, :, 2])
    return cls_score, cls_neg, distance, distance_neg, probs_ori


def kernel(**inputs):
    return _run(inputs, trace=False)


if __name__ == "__main__":
    print("kernel module; use test.py")


# revision 10
# speedup vs baseline: 1.3840x; 1.1672x over previous
"""Trainium2 Bass kernel for nn_DMLNegHead (retrieval_knn head).

Strategy: data-parallel over batch B=16 across 8 NeuronCores (2 images per
core), prototype/offset params replicated. No collectives needed — every
output has a leading batch axis.

Per-core pipeline, processed in spatial chunks of 512 positions:
  conv(1x1) -> PSUM; (emb+bias)^2 on ACT + ones-matmul -> ||emb||^2;
  rnorm = exp(-0.5 ln ssq); partition-broadcast rnorm (GPSIMD);
  emb_n = (emb+bias) * rnorm (DVE scalar_tensor_tensor);
  dot products against 1095 prototype rows, grouped (ori|neg0|neg1) into one
  packed PSUM tile per row-slab so transcendentals run as wide single ACT
  ops, all funcs pinned to the one ACT table set {ln, exp, square}:
     d2 = 2 - 2 dot   (prototypes and emb_n are unit-norm)
     dist = exp(0.5 ln(d2)),  probs_* = exp(affine(...))
  min/mul/add tail on DVE/GPSIMD, class-sum via ones-matmul,
  cls = probs * exp(-ln(sum)).
Outputs are packed ([dist_o|dist_n0|dist_n1], [cls_neg|cls|probs_ori]) and
DMA'd once per two chunks (4 KB contiguous rows) to halve descriptor work.
"""
import sys
sys.path.insert(0, "/opt/trn_rl_repo")

import numpy as np
import concourse.bass as bass
import concourse.tile as tile
from concourse import bacc, mybir, hw_specs
from concourse.bass_utils import run_bass_kernel_spmd

F32 = mybir.dt.float32
F32R = mybir.dt.float32r
AF = mybir.ActivationFunctionType
ALU = mybir.AluOpType

# problem constants (hardcoded per contract)
B, CIN, H, W = 16, 256, 64, 64
E, R, NEG = 256, 365, 2
NCORES = 8
BL = B // NCORES          # batches per core
N = H * W                 # 4096 spatial positions
NCH = 512                 # chunk of positions per inner step
NJ = N // NCH
SIGMA, BETA = 0.5, 0.3
INV2S2 = 1.0 / (2.0 * SIGMA ** 2)   # 2.0

# matmul dtype for conv + dot matmuls: 'f32' (exact, 4 cyc/row) or
# 'f32r' (tf32-like, 1 cyc/row). ssq/sum matmuls stay fp32.
DT_MM_DEFAULT = "f32r"

# slabs of prototype rows: (row0, nrows); 365 = 128 + 128 + 109
SLABS = [(0, 128), (128, 128), (256, R - 256)]

LAST_EXEC_TIME_NS = None

_ACT_SET = "natural_log_exp_and_others"
_ACT_PINNED = False


def _pin_act_tables():
    """Make natural_log_exp_and_others the only candidate set for the
    functions this kernel uses, so the table-load pass emits one load
    instead of ping-ponging between per-function default sets."""
    global _ACT_PINNED
    if _ACT_PINNED:
        return
    tabs = hw_specs.get_activation_tables("gen3")
    pinned = {AF.Exp, AF.Ln, AF.Square}
    assert pinned <= tabs[_ACT_SET]
    for name, funcs in tabs.items():
        if name != _ACT_SET:
            funcs -= pinned
    _ACT_PINNED = True


def _build(dt_mm: str):
    DTM = F32R if dt_mm == "f32r" else F32
    _pin_act_tables()
    nc = bacc.Bacc("TRN2", target_bir_lowering=False)

    x_d = nc.dram_tensor("x", [BL, CIN, N], F32, kind="ExternalInput")
    convT_d = nc.dram_tensor("convT", [CIN, E], F32, kind="ExternalInput")
    convb_d = nc.dram_tensor("convb", [128, 2], F32, kind="ExternalInput")
    repsT_d = nc.dram_tensor("repsT", [E, 3 * R], F32, kind="ExternalInput")

    # o_pack: (dist_o, dist_n0, dist_n1); o_misc: (cls_neg, cls, probs_ori)
    o_pack = nc.dram_tensor("o_pack", [BL, R, 3, N], F32, kind="ExternalOutput")
    o_misc = nc.dram_tensor("o_misc", [BL, R, 3, N], F32, kind="ExternalOutput")

    with tile.TileContext(nc) as tc:
        with (
            tc.tile_pool(name="const", bufs=1) as const,
            tc.tile_pool(name="io", bufs=2) as io,
            tc.tile_pool(name="mid", bufs=2) as mid,
            tc.tile_pool(name="lnp", bufs=2) as lnp,
            tc.tile_pool(name="dpk", bufs=6) as dpk,
            tc.tile_pool(name="ch", bufs=3) as chp,
            tc.tile_pool(name="wp", bufs=6) as wp,
            tc.tile_pool(name="ps_emb", bufs=3, space="PSUM") as ps_emb,
            tc.tile_pool(name="ps_dot", bufs=3, space="PSUM") as ps_dot,
            tc.tile_pool(name="ps_ssq", bufs=2, space="PSUM") as ps_ssq,
        ):
            # ---- resident constants ----
            convT_f = const.tile([128, 2, E], F32)      # [k, ktile, E]
            nc.sync.dma_start(out=convT_f[:],
                              in_=convT_d[:].rearrange("(a k) e -> k a e", k=128))
            convb = const.tile([128, 2], F32)           # per-partition bias, e-tile
            nc.sync.dma_start(out=convb[:], in_=convb_d[:])
            repsT_f = const.tile([128, 2, 3 * R], F32)
            nc.sync.dma_start(out=repsT_f[:],
                              in_=repsT_d[:].rearrange("(a k) r -> k a r", k=128))

            ones_f = const.tile([128, 1], F32)
            nc.vector.memset(ones_f[:], 1.0)
            bias_c = const.tile([128, 1], F32)     # c = en2 + rn2 = 2.0
            nc.vector.memset(bias_c[:], 2.0)
            bias_mc = const.tile([128, 1], F32)    # -INV2S2 * c
            nc.vector.memset(bias_mc[:], -2.0 * INV2S2)

            if DTM is F32R:
                repsT_s = const.tile([128, 2, 3 * R], F32R)
                nc.vector.tensor_copy(repsT_s[:], repsT_f[:])
            else:
                repsT_s = repsT_f

            for b in range(BL):
                for j in range(NJ):
                    ns = slice(j * NCH, (j + 1) * NCH)

                    # -- load x chunk --
                    xs = io.tile([128, 2, NCH], F32, tag="x")
                    nc.sync.dma_start(
                        out=xs[:], in_=x_d[b].rearrange(
                            "(k a) n -> a k n", a=128)[:, :, ns])

                    # -- conv (fp32): emb[e, n] in PSUM (two 1-bank tiles) --
                    embp = []
                    sq = mid.tile([128, 2, NCH], F32, tag="sq")
                    for m in range(2):
                        em = ps_emb.tile([128, NCH], F32, tag="emb", name="emb")
                        mcols = slice(m * 128, (m + 1) * 128)
                        nc.tensor.matmul(em[:], convT_f[:, 0, mcols],
                                         xs[:, 0, :], start=True, stop=False)
                        nc.tensor.matmul(em[:], convT_f[:, 1, mcols],
                                         xs[:, 1, :], start=False, stop=True)
                        embp.append(em)
                        # ssq operand: (emb+b)^2
                        nc.scalar.activation(sq[:, m, :], em[:],
                                             AF.Square, bias=convb[:, m:m + 1])
                    ssq = ps_ssq.tile([1, NCH], F32, tag="ssq")
                    nc.tensor.matmul(ssq[:], ones_f[:], sq[:, 0, :],
                                     start=True, stop=False)
                    nc.tensor.matmul(ssq[:], ones_f[:], sq[:, 1, :],
                                     start=False, stop=True)

                    # -- rnorm = ssq^-0.5 = exp(-0.5 ln(ssq)) --
                    lnssq = mid.tile([1, NCH], F32, tag="lnssq")
                    nc.scalar.activation(lnssq[:], ssq[:], AF.Ln)
                    rnorm = mid.tile([1, NCH], F32, tag="rnorm")
                    nc.scalar.activation(rnorm[:], lnssq[:], AF.Exp, scale=-0.5)
                    bcast = mid.tile([128, NCH], F32, tag="bcast")
                    nc.gpsimd.partition_broadcast(bcast[:], rnorm[:])

                    # -- emb_n = (emb + b) * rnorm --
                    embn = mid.tile([128, 2, NCH], DTM, tag="embn")
                    for m in range(2):
                        nc.vector.scalar_tensor_tensor(
                            out=embn[:, m, :], in0=embp[m][:],
                            scalar=convb[:, m:m + 1], in1=bcast[:],
                            op0=ALU.add, op1=ALU.mult)

                    # -- dots per (slab, group) in 1-bank PSUM tiles --
                    w_t = [None] * 3
                    for s, (r0, p) in enumerate(SLABS):
                        lnd = lnp.tile([128, 3, NCH], F32, tag="lnd")
                        dp0 = None
                        for g in range(3):
                            col0 = g * R + r0
                            dp = ps_dot.tile([128, NCH], F32, tag="dot",
                                             name="dot")
                            nc.tensor.matmul(
                                dp[:p, :], repsT_s[:, 0, col0:col0 + p],
                                embn[:, 0, :], start=True, stop=False)
                            nc.tensor.matmul(
                                dp[:p, :], repsT_s[:, 1, col0:col0 + p],
                                embn[:, 1, :], start=False, stop=True)
                            # d2 = 2 - 2 dot
                            nc.scalar.activation(lnd[:p, g, :], dp[:p, :],
                                                 AF.Ln, bias=bias_c[:p, :],
                                                 scale=-2.0)
                            if g == 0:
                                dp0 = dp

                        # dist = exp(0.5 ln d2)  (dist_o, dist_n0, dist_n1)
                        dist = dpk.tile([128, 3, NCH], F32, tag="dist",
                                        name="dist")
                        nc.scalar.activation(dist[:p, :, :], lnd[:p, :, :],
                                             AF.Exp, scale=0.5)
                        nc.sync.dma_start(out=o_pack[b, r0:r0 + p, :, ns],
                                          in_=dist[:p, :, :])
                        # w slots: 0=cls_neg, 1=probs->cls, 2=probs_ori
                        w = wp.tile([128, 3, NCH], F32, tag="w", name="w")
                        w_t[s] = w
                        nc.scalar.activation(w[:p, 2, :], dp0[:p, :],
                                             AF.Exp, bias=bias_mc[:p, :],
                                             scale=2.0 * INV2S2)

                        dnmin = chp.tile([128, NCH], F32, tag="dnmin")
                        nc.vector.tensor_tensor(
                            dnmin[:p, :], dist[:p, 1, :], dist[:p, 2, :],
                            op=ALU.min)
                        # u slots: 0=dnmin^2, 1=t^2
                        u = chp.tile([128, 2, NCH], F32, tag="u")
                        nc.gpsimd.tensor_mul(u[:p, 0, :], dnmin[:p, :],
                                             dnmin[:p, :])
                        st = chp.tile([128, NCH], F32, tag="st")
                        nc.vector.tensor_scalar(
                            out=st[:p, :], in0=dnmin[:p, :],
                            scalar1=2.0, scalar2=-BETA,
                            op0=ALU.subtract, op1=ALU.mult)
                        t = chp.tile([128, NCH], F32, tag="t")
                        nc.vector.tensor_add(t[:p, :], dist[:p, 0, :],
                                             st[:p, :])
                        nc.vector.tensor_mul(u[:p, 1, :], t[:p, :], t[:p, :])
                        # (cls_neg, probs) = exp(-INV2S2 * (dnmin2, t2))
                        nc.scalar.activation(w[:p, 0:2, :], u[:p, :, :],
                                             AF.Exp, scale=-INV2S2)

                        if s == 0:
                            psum = ps_ssq.tile([1, NCH], F32, tag="ssq",
                                               name="psum")
                        nc.tensor.matmul(psum[:], ones_f[:p, :],
                                         w[:p, 1, :],
                                         start=(s == 0), stop=(s == 2))

                    # cls = probs * exp(-ln(sum))
                    lnsum = mid.tile([1, NCH], F32, tag="lnsum")
                    nc.scalar.activation(lnsum[:], psum[:], AF.Ln)
                    rsum = mid.tile([1, NCH], F32, tag="rsum")
                    nc.scalar.activation(rsum[:], lnsum[:], AF.Exp, scale=-1.0)
                    rbc = mid.tile([128, NCH], F32, tag="rbc")
                    nc.gpsimd.partition_broadcast(rbc[:], rsum[:])
                    for s, (r0, p) in enumerate(SLABS):
                        w = w_t[s]
                        nc.vector.tensor_mul(w[:p, 1, :], w[:p, 1, :],
                                             rbc[:p, :])
                        nc.scalar.dma_start(out=o_misc[b, r0:r0 + p, :, ns],
                                            in_=w[:p, :, :])
    nc.compile()
    return nc


_NC_CACHE = {}


def _host_prep(x, conv_w, conv_b, representations, neg_w, neg_b):
    f = np.float32
    x = np.asarray(x, f)
    conv_w = np.asarray(conv_w, f)
    conv_b = np.asarray(conv_b, f)
    reps = np.asarray(representations, f)
    neg_w = np.asarray(neg_w, f)
    neg_b = np.asarray(neg_b, f)

    r0 = reps[:, 0, :]                                     # [R, E]
    off = (np.abs(r0) @ neg_w.T + neg_b).reshape(R, NEG, E).astype(f)
    rneg = ((off + np.abs(reps)) * np.sign(reps)).astype(f)
    nrm = np.sqrt((rneg * rneg).sum(2, keepdims=True, dtype=f))
    rneg = (rneg / np.maximum(nrm, 1e-12)).astype(f)

    # repsT: [E, 3R] columns = [ori | neg m0 | neg m1]
    allr = np.concatenate([r0[None], rneg[:, 0][None], rneg[:, 1][None]], 0)
    repsT = np.ascontiguousarray(allr.reshape(3 * R, E).T).astype(f)

    convT = np.ascontiguousarray(conv_w.T).astype(f)       # [CIN, E]
    convb2 = np.ascontiguousarray(conv_b.reshape(2, 128).T)  # [128, ktile]

    shared = {"convT": convT, "convb": convb2, "repsT": repsT}
    in_maps = []
    for i in range(NCORES):
        m = dict(shared)
        m["x"] = np.ascontiguousarray(
            x[i * BL:(i + 1) * BL].reshape(BL, CIN, N))
        in_maps.append(m)
    return in_maps


def _run(inputs, dt_mm=DT_MM_DEFAULT, trace=False):
    global LAST_EXEC_TIME_NS
    in_maps = _host_prep(**inputs)
    if dt_mm not in _NC_CACHE:
        _NC_CACHE[dt_mm] = _build(dt_mm)
    nc = _NC_CACHE[dt_mm]
    res = run_bass_kernel_spmd(nc, in_maps, list(range(NCORES)), trace=trace)
    LAST_EXEC_TIME_NS = res.exec_time_ns

    def cat(name):
        return np.concatenate([res.results[i][name] for i in range(NCORES)], 0)

    pack = cat("o_pack").reshape(B, R, 3, H, W)
    misc = cat("o_misc").reshape(B, R, 3, H, W)
    distance = np.ascontiguousarray(pack[:, :, 0:1])
    distance_neg = np.ascontiguousarray(pack[:, :, 1:3])
    cls_neg = np.ascontiguousarray(misc[:, :, 0])
    cls_score = np.ascontiguousarray(misc[:, :, 1])
    probs_ori = np.ascontiguousarray(misc[:, :, 2])
    return cls_score, cls_neg, distance, distance_neg, probs_ori


def kernel(**inputs):
    return _run(inputs, trace=False)


if __name__ == "__main__":
    print("kernel module; use test.py")
